# revision 21
# baseline (speedup 1.0000x reference)
"""CLAHE-3D Trainium2 kernel (Bass/Tile, 8-core SPMD).

Pipeline (all on device; host only shards inputs, concatenates outputs):
  phase 1: per-tile Gaussian-KDE histograms.  Bins live on partitions
           (2 tiles x 64 bins = 128 partitions); the voxel tile is
           PE-broadcast to all partitions, then two ACT passes
           (Square with per-partition bias, Exp with accum_out) produce
           exact reference wk sums per (tile, bin).
  phase 2: AllGather raw histograms (16KB/core), then every core runs the
           tiny clip/redistribute/cumsum on all 512 tiles -> cdf[512, 64].
  phase 3: separable spline interpolation as PE matmuls:
           stage 1: per bin b, U1[(i,j), w] = sum_k cdf[ijk,b] * Mw[w,k]
           stage 2 (per h-octet block): S[(d,h8), (w,b)] =
                   sum_{ij} (Md[d,i]*Mh[h,j]) * U1[(ij), (w,b)]
  phase 4: per-voxel 6-tap quintic bin interpolation WITHOUT gather:
           S rows are reflect-padded to 74-wide segments (S_ext); for each
           tap t a masked-reset tensor_tensor_scan (state = maskinv*state+S)
           yields the suffix sum from bin (m+t); consecutive suffix ends
           differ by exactly the gathered tap value S_ext[m+t].  Tap weights
           are the closed-form single-piece quintics of reference bspline5.
  finale:  global min/max via AllReduce, on-device normalization, f16 out.

The d-axis is sharded: core r owns d-planes [16r, 16r+16) == tile row i=r.

Host<->device traffic is the dominant cost on the axon-tunneled PJRT path
(~60 MB/s wire), so x ships once per core in its natural [16,128,128]
layout (both the KDE tiling and the h-octet blocking are derived on device
via strided DMA), the stage-2 LHS (Md x Mh blocks) is built on device from
tiny factors, broadcast constants ship as single rows, and the output is
f16.  The JAX persistent compilation cache removes the per-dispatch NEFF
recompile that run_bass_via_pjrt's fresh-jit-per-call otherwise triggers.
"""

import os
import sys

import numpy as np

sys.path.insert(0, "/opt/trn_rl_repo")

import jax

for _k, _v in (
    ("jax_compilation_cache_dir", os.path.join("/tmp", "clahe3d_jax_cache")),
    ("jax_persistent_cache_min_entry_size_bytes", -1),
    ("jax_persistent_cache_min_compile_time_secs", 0.0),
):
    try:
        jax.config.update(_k, _v)
    except Exception:
        pass

import concourse.bacc as bacc
import concourse.bass as bass
import concourse.mybir as mybir
import concourse.tile as tile
from concourse.bass_utils import run_bass_kernel_spmd

F32 = mybir.dt.float32
F16 = mybir.dt.float16
U16 = mybir.dt.uint16
U8 = mybir.dt.uint8
AF = mybir.ActivationFunctionType
ALU = mybir.AluOpType
AX = mybir.AxisListType

N_CORES = 8
D = H = W = 128
GD = GH = GW = 8
TD = TH = TW = 16
VPT = TD * TH * TW            # 4096
NB = 64
DS = D // N_CORES             # 16 d-planes per core
NT_OWN = GH * GW              # 64 tiles per core
NPAIR = NT_OWN // 2           # 32 tile pairs in phase 1
BW_KDE = 0.001
EXTW = 74                     # padded S segment width (2+64+2 used, 6 zero)
NSEG = W                      # 128 segments (one per w) per partition
SCAN_N = NSEG * EXTW          # 9472 scanned elements
NBLK = 16                     # h-octet blocks
LIMIT = float(np.floor(4.0 * VPT / NB))   # 256.0


# ----------------------------------------------------------------------------
# host-side constants (float32, mirrors reference.axis_matrix)
# ----------------------------------------------------------------------------
def _bspline5_np(x):
    t = np.abs(np.asarray(x, np.float64))
    w0 = 11.0 / 20.0 - t**2 / 2.0 + t**4 / 4.0 - t**5 / 12.0
    w1 = (17.0 / 40.0 + 5.0 * t / 8.0 - 7.0 * t**2 / 4.0 + 5.0 * t**3 / 4.0
          - 3.0 * t**4 / 8.0 + t**5 / 24.0)
    w2 = (3.0 - t) ** 5 / 120.0
    return np.where(t < 1.0, w0, np.where(t < 2.0, w1, np.where(t < 3.0, w2, 0.0)))


def _axis_matrix_np(size, g):
    c = np.linspace(-0.5 - 0.25 / g, g - 1 + 0.5 + 0.25 / g, size, dtype=np.float32)
    base = np.floor(c).astype(np.int32) - 2
    taps = base[:, None] + np.arange(6)[None, :]
    wgt = _bspline5_np(c[:, None].astype(np.float32)
                       - taps.astype(np.float32)).astype(np.float32)
    i = np.remainder(taps, 2 * g)
    idx = np.where(i < g, i, 2 * g - 1 - i)
    M = np.zeros((size, g), np.float32)
    np.add.at(M, (np.arange(size)[:, None].repeat(6, 1), idx), wgt)
    return M


def _host_constants():
    # D == H == W == 128 and GD == GH == GW == 8, so one axis matrix serves
    # all three axes.
    M = _axis_matrix_np(D, GD)
    mT = np.ascontiguousarray(M.T).astype(np.float32)          # [8, 128]

    consts = {}
    # ohJ[h, j] = [h // 16 == j]: PE partition-reduction of the per-plane
    # KDE partial sums over each tile's 16 h-rows
    ohJ = np.zeros((128, 8), np.float32)
    ohJ[np.arange(128), np.arange(128) // 16] = 1.0
    consts["ohJ"] = ohJ

    # one-hot selectors for the on-device LHS (Kronecker) build:
    #   ohA[k, (i,j)] = [i == k],  ohB[k, (i,j)] = [j == k]
    ii, jj = np.divmod(np.arange(64), 8)
    ohA = np.zeros((8, 64), np.float32)
    ohA[ii, np.arange(64)] = 1.0
    ohB = np.zeros((8, 64), np.float32)
    ohB[jj, np.arange(64)] = 1.0

    # quintic tap-weight coefficients (Horner, highest power first), per tap:
    #   t=0: B5(f+2) = (1-f)^5/120      t=3: B5(1-f)   (w0 piece)
    #   t=1: B5(f+1) (w1 piece)         t=4: B5(2-f)   (w1 piece)
    #   t=2: B5(f)   (w0 piece)         t=5: B5(f-3) = f^5/120
    def poly_from(fn):
        xs = np.linspace(0.0, 1.0, 6)
        V = np.vander(xs, 6, increasing=True)
        c = np.linalg.solve(V, fn(xs))
        return c[::-1]

    polys = [
        poly_from(lambda f: _bspline5_np(f + 2.0)),
        poly_from(lambda f: _bspline5_np(f + 1.0)),
        poly_from(lambda f: _bspline5_np(f)),
        poly_from(lambda f: _bspline5_np(1.0 - f)),
        poly_from(lambda f: _bspline5_np(2.0 - f)),
        poly_from(lambda f: _bspline5_np(f - 3.0)),
    ]
    coef = np.stack(polys, 1).astype(np.float32)          # [6 deg, 6 tap]

    # rows[0, 0:64]  = iota64, rows[0, 64:100] = wb coefs,
    # rows[0, 100:174] = iota74
    rows = np.zeros((1, 174), np.float32)
    rows[0, 0:NB] = np.arange(NB, dtype=np.float32)
    rows[0, NB:NB + 36] = coef.reshape(36)
    rows[0, 100:100 + EXTW] = np.arange(EXTW, dtype=np.float32)
    consts["rows"] = rows

    # m8 packs the [8, .] family: [0:16]=mdT (per core), [16:80]=ohA,
    # [80:144]=ohB, [144:272]=mT
    m8_all = []
    for r in range(N_CORES):
        m8 = np.zeros((8, 272), np.float32)
        m8[:, 0:DS] = mT[:, r * DS:(r + 1) * DS]
        m8[:, 16:80] = ohA
        m8[:, 80:144] = ohB
        m8[:, 144:272] = mT
        m8_all.append(m8)
    return consts, m8_all


# ----------------------------------------------------------------------------
# the Bass program (SPMD; identical on all cores, per-core data via inputs)
# ----------------------------------------------------------------------------
def _build_program():
    nc = bacc.Bacc("TRN2", target_bir_lowering=False, debug=False,
                   num_devices=N_CORES)

    # per-core voxel shard, natural layout
    x_in = nc.dram_tensor("x", [DS, H, W], U16, kind="ExternalInput")
    # output in kernel layout [(d,h8), (blk, w)], u8; host un-permutes
    y_out = nc.dram_tensor("y", [128, NBLK * W], U8, kind="ExternalOutput")

    ohJ_d = nc.dram_tensor("ohJ", [128, 8], F32, kind="ExternalInput")
    m8_d = nc.dram_tensor("m8", [8, 272], F32, kind="ExternalInput")
    rows_d = nc.dram_tensor("rows", [1, 174], F32, kind="ExternalInput")

    s_act = float(np.float32(1.0) / np.float32(BW_KDE))

    with tile.TileContext(nc) as tc:
        with (
            tc.tile_pool(name="const", bufs=1) as cpool,
            tc.tile_pool(name="dram", bufs=1, space="DRAM") as dpool,
            tc.tile_pool(name="p1", bufs=2) as p1,
            tc.tile_pool(name="p1ps", bufs=2, space="PSUM") as p1ps,
            tc.tile_pool(name="small", bufs=2) as sm,
            tc.tile_pool(name="u1ps", bufs=2, space="PSUM") as u1ps,
            tc.tile_pool(name="big", bufs=1) as big,
            tc.tile_pool(name="scan", bufs=1) as scanp,
            tc.tile_pool(name="sx", bufs=1) as sxp,
            tc.tile_pool(name="blk", bufs=2) as blkp,
            tc.tile_pool(name="s2ps", bufs=2, space="PSUM") as s2ps,
        ):
            # ---- collective bounce buffers -------------------------------
            hist_own = dpool.tile([NT_OWN, NB], F32, name="hist_own")
            hist_all = dpool.tile([N_CORES * NT_OWN, NB], F32,
                                  addr_space="Shared", name="hist_all")
            cdf_dram = dpool.tile([512, NB], F32, name="cdf_dram")
            mm_in = dpool.tile([1, 4], F32, name="mm_in")
            mm_min = dpool.tile([1, 4], F32, addr_space="Shared", name="mm_min")
            mm_max = dpool.tile([1, 4], F32, addr_space="Shared", name="mm_max")
            sb_dram = dpool.tile([1, 2], F32, name="sb_dram")

            # ---- constants ----------------------------------------------
            c_ohJ = cpool.tile([128, 8], F32)
            nc.sync.dma_start(c_ohJ[:], ohJ_d[:])
            c_m8 = cpool.tile([8, 272], F32)
            nc.sync.dma_start(c_m8[:], m8_d[:])
            c_mdT = c_m8[:, 0:DS]
            c_ohA = c_m8[:, 16:80]
            c_ohB = c_m8[:, 80:144]
            c_mT = c_m8[:, 144:272]
            c_rows = cpool.tile([128, 174], F32)
            nc.sync.dma_start(c_rows[:], rows_d[:].broadcast_to([128, 174]))
            c_iota64 = c_rows[:, 0:NB]
            c_wbcoef = c_rows[:, NB:NB + 36]
            c_iota74f = c_rows[:, 100:100 + EXTW]

            # ---- on-device LHS build: c_lhs[(ij), (blk,d,h8)] -----------
            #   = Md[16r+d, i] * Mh[blk*8+h8, j]
            c_ma = cpool.tile([8, 128], F32)        # ma[i,(d,h8)] = mdT[i,d]
            nc.scalar.copy(
                c_ma[:].rearrange("p (d h) -> p d h", h=8),
                c_mdT.unsqueeze(2).broadcast_to([8, DS, 8]))
            a_ps = u1ps.tile([64, 128], F32, tag="u1ps", space="PSUM")
            nc.tensor.matmul(a_ps[:], c_ohA, c_ma[:], start=True, stop=True)
            a_sb = cpool.tile([64, 128], F32)       # A[(ij),(d,h8)] = Md[.,i]
            nc.scalar.copy(a_sb[:], a_ps[:])
            b_ps = u1ps.tile([64, 128], F32, tag="u1ps", space="PSUM")
            nc.tensor.matmul(b_ps[:], c_ohB, c_mT, start=True, stop=True)
            b_sb = cpool.tile([64, 128], F32)       # B[(ij),h] = Mh[h,j]
            nc.scalar.copy(b_sb[:], b_ps[:])
            c_lhs = cpool.tile([64, NBLK * 128], F32)
            c_lhs_v = c_lhs[:].rearrange("p (n m) -> p n m", n=NBLK)
            a_v = a_sb[:].rearrange("p (d h) -> p d h", h=8)
            for blk in range(NBLK):
                nc.vector.tensor_tensor(
                    c_lhs_v[:, blk:blk + 1, :].squeeze(1)
                    .rearrange("p (d h) -> p d h", h=8),
                    a_v,
                    b_sb[:, blk * 8:(blk + 1) * 8].unsqueeze(1)
                    .broadcast_to([64, DS, 8]),
                    op=ALU.mult)

            # ---- stage xblocks in DRAM: [(d,h8), (blk,w)] u16 ----------
            xb_dram = dpool.tile([128, NBLK * W], U16, name="xb_st")
            for d in range(DS):
                nc.sync.dma_start(
                    xb_dram[d * 8:(d + 1) * 8, :]
                    .rearrange("h (n w) -> h n w", n=NBLK),
                    x_in[d:d + 1, :, :]
                    .rearrange("o (n h) w -> o n h w", n=NBLK)
                    .squeeze(0).transpose([1, 0, 2]))

            # ---- phase 1: KDE histograms, voxels on h-partitions --------
            # per d-plane: diff[h,(k,b,w)] = x[h,(k,w)] - bin_b  (f32 math,
            # f16 result), then f16 Square/Exp on ACT, w-reduce to f32,
            # accumulate over d; finally PE-reduce h-groups via ohJ.
            c_binsf = cpool.tile([128, NB], F32)
            nc.vector.tensor_scalar(c_binsf[:], c_iota64,
                                    1.0 / float(NB - 1), None, op0=ALU.mult)
            acc1 = p1.tile([128, GW * NB], F32, tag="kacc", bufs=1)
            nc.vector.memset(acc1[:], 0.0)
            KG = 2                                       # k-tiles per group
            for d in range(DS):
                xpu = p1.tile([128, W], U16, tag="xpu")
                nc.sync.dma_start(xpu[:], x_in[d:d + 1, :, :].squeeze(0))
                xf = p1.tile([128, W], F32, tag="xf")
                nc.vector.tensor_scalar(xf[:], xpu[:], 1.0 / 65535.0, None,
                                        op0=ALU.mult)
                for g in range(GW // KG):
                    diff = p1.tile([128, KG * NB * TW], F16, tag="kdiff",
                                   bufs=1)
                    dv = diff[:].rearrange("p (k b w) -> p k b w", k=KG, b=NB)
                    nc.vector.tensor_tensor(
                        dv,
                        xf[:, g * KG * TW:(g + 1) * KG * TW]
                        .rearrange("p (k w) -> p k w", k=KG)
                        .unsqueeze(2).broadcast_to([128, KG, NB, TW]),
                        c_binsf[:].unsqueeze(1).broadcast_to([128, KG, NB])
                        .unsqueeze(3).broadcast_to([128, KG, NB, TW]),
                        op=ALU.subtract)
                    # scale/32 keeps the f16 square finite (<=977);
                    # the 32^2 is folded into the Exp scale
                    nc.scalar.activation(diff[:], diff[:], AF.Square,
                                         bias=0.0, scale=31.25)
                    nc.scalar.activation(diff[:], diff[:], AF.Exp,
                                         bias=0.0, scale=-512.0)
                    red = p1.tile([128, KG * NB], F32, tag="kred", bufs=1)
                    nc.vector.tensor_reduce(
                        red[:].rearrange("p (k b) -> p k b", k=KG),
                        dv, axis=AX.X, op=ALU.add)
                    nc.vector.tensor_tensor(
                        acc1[:, g * KG * NB:(g + 1) * KG * NB],
                        acc1[:, g * KG * NB:(g + 1) * KG * NB],
                        red[:], op=ALU.add)
            hist_ps = p1ps.tile([8, GW * NB], F32, tag="histps", bufs=1,
                                space="PSUM")
            nc.tensor.matmul(hist_ps[:], c_ohJ[:], acc1[:],
                             start=True, stop=True)
            hist_sb2 = sm.tile([8, GW * NB], F32, tag="hist2")
            nc.scalar.copy(hist_sb2[:], hist_ps[:])
            nc.sync.dma_start(
                hist_own[:].rearrange("(j k) b -> j k b", k=GW),
                hist_sb2[:].rearrange("p (k b) -> p k b", k=GW))

            # ---- AllGather ----------------------------------------------
            nc.gpsimd.collective_compute(
                "AllGather", ALU.bypass,
                replica_groups=[list(range(N_CORES))],
                ins=[hist_own[:]], outs=[hist_all[:]])

            # ---- phase 2: clip/redistribute/cdf (all 512 tiles) ---------
            for chunk in range(4):
                hh = sm.tile([128, NB], F32, tag="ph2h")
                nc.sync.dma_start(hh[:],
                                  hist_all[chunk * 128:(chunk + 1) * 128, :])
                ssum = sm.tile([128, 1], F32, tag="ph2s")
                nc.vector.tensor_reduce(ssum[:], hh[:], axis=AX.X, op=ALU.add)
                denom = sm.tile([128, 1], F32, tag="ph2d")
                nc.vector.tensor_scalar(denom[:], ssum[:], 1.0 / VPT, 1e-10,
                                        op0=ALU.mult, op1=ALU.add)
                dinv = sm.tile([128, 1], F32, tag="ph2di")
                nc.vector.reciprocal(dinv[:], denom[:])
                nc.vector.tensor_scalar(hh[:], hh[:], dinv[:], LIMIT,
                                        op0=ALU.mult, op1=ALU.min)
                clip = sm.tile([128, 1], F32, tag="ph2c")
                nc.vector.tensor_reduce(clip[:], hh[:], axis=AX.X, op=ALU.add)
                nc.vector.tensor_scalar(clip[:], clip[:], -1.0, float(VPT),
                                        op0=ALU.mult, op1=ALU.add)
                qq = sm.tile([128, 1], F32, tag="ph2q")
                nc.vector.tensor_scalar(qq[:], clip[:], 1.0 / NB, None,
                                        op0=ALU.mult)
                rq = sm.tile([128, 1], F32, tag="ph2rq")
                nc.vector.tensor_scalar(rq[:], qq[:], 8388608.0, 8388608.0,
                                        op0=ALU.add, op1=ALU.subtract)
                ltq = sm.tile([128, 1], F32, tag="ph2ltq")
                nc.vector.tensor_tensor(ltq[:], qq[:], rq[:], op=ALU.is_lt)
                redist = sm.tile([128, 1], F32, tag="ph2rd")
                nc.vector.tensor_tensor(redist[:], rq[:], ltq[:],
                                        op=ALU.subtract)
                rs64 = sm.tile([128, 1], F32, tag="ph2r64")
                nc.vector.tensor_scalar(rs64[:], redist[:], float(NB), None,
                                        op0=ALU.mult)
                resid = sm.tile([128, 1], F32, tag="ph2r")
                nc.vector.tensor_tensor(resid[:], clip[:], rs64[:],
                                        op=ALU.subtract)
                nc.vector.tensor_scalar(hh[:], hh[:], redist[:], None,
                                        op0=ALU.add)
                lt = sm.tile([128, NB], F32, tag="ph2lt")
                nc.vector.tensor_scalar(lt[:], c_iota64, resid[:], None,
                                        op0=ALU.is_lt)
                nc.vector.tensor_tensor(hh[:], hh[:], lt[:], op=ALU.add)
                zero1 = sm.tile([128, NB], F32, tag="ph2z")
                nc.vector.memset(zero1[:], 0.0)
                cs = sm.tile([128, NB], F32, tag="ph2cs")
                nc.vector.tensor_tensor_scan(cs[:], hh[:], zero1[:], 0.0,
                                             op0=ALU.add, op1=ALU.add)
                nc.vector.tensor_scalar(cs[:], cs[:], float(NB - 1) / VPT,
                                        None, op0=ALU.mult)
                nc.sync.dma_start(cdf_dram[chunk * 128:(chunk + 1) * 128, :],
                                  cs[:])

            # ---- phase 3 stage 1: U1[(ij), (w,b)] -----------------------
            cdf2 = sm.tile([8, 64 * NB], F32, tag="cdf2")
            nc.sync.dma_start(
                cdf2[:].rearrange("p (ij b) -> p ij b", ij=64),
                cdf_dram[:].rearrange("(ij k) b -> k ij b", k=8))
            cdf2v = cdf2[:].rearrange("p (ij b) -> p ij b", ij=64)
            u1 = big.tile([64, W * NB], F32, tag="u1")
            u1v = u1[:].rearrange("p (w b) -> p w b", b=NB)
            for b0 in range(0, NB, 4):
                ps = u1ps.tile([64, 4 * W], F32, tag="u1ps", space="PSUM")
                psv = ps[:].rearrange("p (q w) -> p q w", q=4)
                for q in range(4):
                    nc.tensor.matmul(psv[:, q:q + 1, :].squeeze(1),
                                     cdf2v[:, :, b0 + q:b0 + q + 1].squeeze(2),
                                     c_mT, start=True, stop=True)
                nc.scalar.copy(u1v[:, :, b0:b0 + 4],
                               psv.transpose([0, 2, 1]))

            # ---- phase 3 stage 2 + phase 4, per h-octet block -----------
            omin = sm.tile([128, 1], F32, tag="omin")
            omax = sm.tile([128, 1], F32, tag="omax")
            yall = big.tile([128, NBLK * W], F32, tag="yall")

            for blk in range(NBLK):
                sext = sxp.tile([128, (NSEG + 1) * EXTW], F32, tag="sext")
                sxv = sext[:].rearrange("p (w e) -> p w e", e=EXTW)
                nc.vector.memset(sxv[:, :, 68:EXTW], 0.0)
                nc.vector.memset(sxv[:, NSEG:NSEG + 1, :], 0.0)
                for ch in range(16):
                    ps2 = s2ps.tile([128, 512], F32, tag="s2", space="PSUM")
                    nc.tensor.matmul(ps2[:],
                                     c_lhs_v[:, blk:blk + 1, :].squeeze(1),
                                     u1[:, ch * 512:(ch + 1) * 512],
                                     start=True, stop=True)
                    dst = sxv[:, ch * 8:(ch + 1) * 8, 2:66]
                    nc.scalar.copy(dst,
                                   ps2[:].rearrange("p (w b) -> p w b", b=NB))
                # reflect pad: ext0=S[1],ext1=S[0],ext66=S[63],ext67=S[62]
                nc.scalar.copy(sxv[:, 0:NSEG, 0:1], sxv[:, 0:NSEG, 3:4])
                nc.scalar.copy(sxv[:, 0:NSEG, 1:2], sxv[:, 0:NSEG, 2:3])
                nc.scalar.copy(sxv[:, 0:NSEG, 66:67], sxv[:, 0:NSEG, 65:66])
                nc.scalar.copy(sxv[:, 0:NSEG, 67:68], sxv[:, 0:NSEG, 64:65])

                xb = blkp.tile([128, W], U16, tag="xb", bufs=1)
                nc.sync.dma_start(xb[:], xb_dram[:, blk * W:(blk + 1) * W])
                cb = blkp.tile([128, W], F32, tag="cb", bufs=1)
                nc.vector.tensor_scalar(cb[:], xb[:],
                                        float(NB - 1) / 65535.0, None,
                                        op0=ALU.mult)
                rr = blkp.tile([128, W], F32, tag="rr", bufs=1)
                nc.vector.tensor_scalar(rr[:], cb[:], 8388608.0, 8388608.0,
                                        op0=ALU.add, op1=ALU.subtract)
                ltc = blkp.tile([128, W], F32, tag="ltc", bufs=1)
                nc.vector.tensor_tensor(ltc[:], cb[:], rr[:], op=ALU.is_lt)
                mm = blkp.tile([128, W], F32, tag="mm", bufs=1)
                nc.vector.tensor_tensor(mm[:], rr[:], ltc[:], op=ALU.subtract)
                fr = blkp.tile([128, W], F32, tag="fr", bufs=1)
                nc.vector.tensor_tensor(fr[:], cb[:], mm[:], op=ALU.subtract)
                m6 = blkp.tile([128, W], F32, tag="m6", bufs=1)
                nc.vector.tensor_scalar(m6[:], mm[:], 6.0, None, op0=ALU.add)

                # maskinv[w, q] = (iota_q != m_w + 6), fp16, padded segment
                mask = blkp.tile([128, (NSEG + 1) * EXTW], F16, tag="mask", bufs=1)
                mkv = mask[:].rearrange("p (w e) -> p w e", e=EXTW)
                nc.gpsimd.memset(mkv[:, NSEG:NSEG + 1, :], 1.0)
                nc.vector.tensor_tensor(
                    mkv[:, 0:NSEG, :],
                    c_iota74f.unsqueeze(1).broadcast_to([128, NSEG, EXTW]),
                    m6[:].unsqueeze(2).broadcast_to([128, W, EXTW]),
                    op=ALU.not_equal)

                # 7 masked-reset scans; suffix ends at segment index 73
                tend = blkp.tile([128, 7 * W], F32, tag="tend", bufs=1)
                tview = tend[:].rearrange("p (t w) -> p t w", t=7)
                sbuf = scanp.tile([128, SCAN_N], F32, tag="scanbuf")
                for t in range(7):
                    nc.vector.tensor_tensor_scan(
                        sbuf[:, 0:SCAN_N],
                        mask[:, 6 - t:6 - t + SCAN_N],
                        sext[:, 0:SCAN_N],
                        0.0, op0=ALU.mult, op1=ALU.add)
                    nc.scalar.copy(
                        tview[:, t:t + 1, :],
                        sbuf[:].rearrange("p (w e) -> p w e", e=EXTW)
                        [:, 0:NSEG, 73:74].transpose([0, 2, 1]))

                # taps (6) and quintic weights, batched [128, 6, W]
                taps = blkp.tile([128, 6 * W], F32, tag="taps", bufs=1)
                tp = taps[:].rearrange("p (t w) -> p t w", t=6)
                nc.vector.tensor_tensor(tp, tview[:, 0:6, :],
                                        tview[:, 1:7, :], op=ALU.subtract)
                wbt = blkp.tile([128, 6 * W], F32, tag="wbt", bufs=1)
                wv = wbt[:].rearrange("p (t w) -> p t w", t=6)
                cview = c_wbcoef.rearrange("p (deg t) -> p deg t", deg=6)
                frb = fr[:].unsqueeze(1).broadcast_to([128, 6, W])
                for deg in range(6):
                    coefb = cview[:, deg:deg + 1, :].transpose(
                        [0, 2, 1]).broadcast_to([128, 6, W])
                    if deg == 0:
                        nc.vector.tensor_copy(wv, coefb)
                    else:
                        nc.vector.tensor_tensor(wv, wv, frb, op=ALU.mult)
                        nc.vector.tensor_tensor(wv, wv, coefb, op=ALU.add)
                nc.vector.tensor_tensor(tp, tp, wv, op=ALU.mult)
                # sum 6 taps -> out block (strided reduce over the tap dim)
                acc = yall[:, blk * W:(blk + 1) * W]
                nc.vector.tensor_reduce(
                    acc.unsqueeze(2),
                    taps[:].rearrange("p (t w) -> p w t", t=6),
                    axis=AX.X, op=ALU.add)

            # ---- global min / max ---------------------------------------
            nc.vector.tensor_reduce(omin[:], yall[:], axis=AX.X, op=ALU.min)
            nc.vector.tensor_reduce(omax[:], yall[:], axis=AX.X, op=ALU.max)
            gmin = sm.tile([1, 1], F32, tag="gmin")
            gmax = sm.tile([1, 1], F32, tag="gmax")
            negmin = sm.tile([128, 1], F32, tag="negmin")
            nc.vector.tensor_scalar(negmin[:], omin[:], -1.0, None,
                                    op0=ALU.mult)
            nc.gpsimd.tensor_reduce(gmin[:], negmin[:], axis=AX.XYZWC,
                                    op=ALU.max)
            nc.vector.tensor_scalar(gmin[:], gmin[:], -1.0, None,
                                    op0=ALU.mult)
            nc.gpsimd.tensor_reduce(gmax[:], omax[:], axis=AX.XYZWC,
                                    op=ALU.max)
            g4 = sm.tile([1, 4], F32, tag="g4")
            nc.vector.tensor_copy(g4[:], gmin[:].broadcast_to([1, 4]))
            nc.sync.dma_start(mm_in[:], g4[:])
            nc.gpsimd.collective_compute(
                "AllReduce", ALU.min,
                replica_groups=[list(range(N_CORES))],
                ins=[mm_in[:]], outs=[mm_min[:]])
            g4b = sm.tile([1, 4], F32, tag="g4b")
            nc.vector.tensor_copy(g4b[:], gmax[:].broadcast_to([1, 4]))
            nc.sync.dma_start(mm_in[:], g4b[:])
            nc.gpsimd.collective_compute(
                "AllReduce", ALU.max,
                replica_groups=[list(range(N_CORES))],
                ins=[mm_in[:]], outs=[mm_max[:]])

            # normalize: v*inv + (-mn*inv), write f16
            nmn = sm.tile([1, 4], F32, tag="nmn")
            nmx = sm.tile([1, 4], F32, tag="nmx")
            nc.sync.dma_start(nmn[:], mm_min[:])
            nc.sync.dma_start(nmx[:], mm_max[:])
            rng = sm.tile([1, 1], F32, tag="rng")
            nc.vector.tensor_tensor(rng[:], nmx[:, 0:1], nmn[:, 0:1],
                                    op=ALU.subtract)
            nc.vector.tensor_scalar(rng[:], rng[:], 1e-10, None, op0=ALU.add)
            inv = sm.tile([1, 1], F32, tag="inv")
            nc.vector.reciprocal(inv[:], rng[:])
            nbias = sm.tile([1, 1], F32, tag="nbias")
            nc.vector.tensor_tensor(nbias[:], nmn[:, 0:1], inv[:],
                                    op=ALU.mult)
            nc.vector.tensor_scalar(nbias[:], nbias[:], -1.0, None,
                                    op0=ALU.mult)
            sb2 = sm.tile([1, 2], F32, tag="sb2")
            nc.vector.tensor_copy(sb2[:, 0:1], inv[:])
            nc.vector.tensor_copy(sb2[:, 1:2], nbias[:])
            nc.sync.dma_start(sb_dram[:], sb2[:])
            scal_b = sm.tile([128, 2], F32, tag="scalb")
            nc.sync.dma_start(scal_b[:], sb_dram[:].broadcast_to([128, 2]))
            # stage the normalized volume in the (now dead) mask slot,
            # then clamp to [0, 255] and emit u8 — no new SBUF
            yh = blkp.tile([128, (NSEG + 1) * EXTW], F16, tag="mask", bufs=1)
            nc.scalar.activation(yh[:, 0:NBLK * W], yall[:], AF.Identity,
                                 bias=scal_b[:, 1:2], scale=scal_b[:, 0:1])
            nc.vector.tensor_scalar(yh[:, 0:NBLK * W], yh[:, 0:NBLK * W],
                                    255.0, 255.0, op0=ALU.mult, op1=ALU.min)
            yu8 = blkp.tile([128, 512], U8, tag="yu8", bufs=1)
            for qtr in range(4):
                nc.vector.tensor_scalar(yu8[:], yh[:, qtr * 512:(qtr + 1) * 512],
                                        0.0, None, op0=ALU.max)
                nc.sync.dma_start(y_out[:, qtr * 512:(qtr + 1) * 512], yu8[:])

    nc.compile()
    return nc


_PROGRAM_CACHE = {}


def _get_program():
    if "nc" not in _PROGRAM_CACHE:
        _PROGRAM_CACHE["consts"], _PROGRAM_CACHE["m8"] = _host_constants()
        _PROGRAM_CACHE["nc"] = _build_program()
    return (_PROGRAM_CACHE["nc"], _PROGRAM_CACHE["consts"],
            _PROGRAM_CACHE["m8"])


def _prep_in_maps(xv, consts, m8_all):
    xq = (xv * np.float32(65535.0) + np.float32(0.5)).astype(np.uint16)
    in_maps = []
    for r in range(N_CORES):
        m = {"x": xq[r * DS:(r + 1) * DS], "m8": m8_all[r]}
        m.update(consts)
        in_maps.append(m)
    return in_maps


def _unshard_output(res):
    shards = []
    for r in range(N_CORES):
        yr = (res.results[r]["y"].astype(np.float32)
              * np.float32(1.0 / 255.0)).reshape(DS, 8, NBLK, W)
        shards.append(yr.transpose(0, 2, 1, 3).reshape(DS, H, W))
    return np.concatenate(shards, axis=0)


def kernel(**inputs):
    x = np.asarray(inputs["x"], np.float32)
    orig_shape = x.shape
    xv = np.ascontiguousarray(x.reshape(D, H, W))

    nc, consts, m8_all = _get_program()
    in_maps = _prep_in_maps(xv, consts, m8_all)
    for attempt in range(3):
        res = run_bass_kernel_spmd(nc, in_maps, core_ids=list(range(N_CORES)))
        out = _unshard_output(res)
        # guard against transient device-state garbage: output must be a
        # finite, normalized volume
        if np.isfinite(out).all() and -0.01 <= out.min() and out.max() <= 1.01:
            break
    return out.reshape(orig_shape).astype(np.float32)


if __name__ == "__main__":
    rng = np.random.default_rng(0)
    x = rng.random((1, 1, D, H, W), dtype=np.float32)
    y = kernel(x=x)
    print("kernel ran; out shape", y.shape, "range", y.min(), y.max())


# revision 26
# speedup vs baseline: 1.3959x; 1.3959x over previous
"""CLAHE-3D Trainium2 kernel (Bass/Tile, 8-core SPMD).

Pipeline (all on device; host only shards inputs, concatenates outputs):
  phase 1: per-tile Gaussian-KDE histograms.  Voxels stay on their h-row
           partitions; per d-plane the DVE forms diff[h,(k,b,w)] =
           x[h,(k,w)] - bin_b (f32 math, f16 result), ACT squares and
           exponentiates in f16 (scale split 31.25 / -512 keeps the square
           finite in f16), a w-reduce + d-accumulate gives per-plane sums,
           and one PE matmul with a one-hot [128,8] reduces h-groups to the
           64 per-tile histograms.
  phase 2: AllGather raw histograms (16KB/core), then every core runs the
           tiny clip/redistribute/cumsum on all 512 tiles -> cdf[512, 64].
  phase 3: separable quintic-spline interpolation as PE matmuls:
           stage 1: per bin b, U1[(i,j), w] = sum_k cdf[ijk,b] * Mw[w,k]
           stage 2 (per h-octet block): S[(d,h8), (w,b)] =
                   sum_{ij} (Md[d,i]*Mh[h,j]) * U1[(ij), (w,b)]
           The stage-2 LHS (Md x Mh Kronecker blocks) is built on device
           from tiny shipped factors via two one-hot PE broadcasts and a
           broadcast multiply.
  phase 4: per-voxel 6-tap quintic bin interpolation WITHOUT gather:
           S rows are reflect-padded to 74-wide segments (S_ext); for each
           tap t a masked-reset tensor_tensor_scan (state = maskinv*state+S)
           yields the suffix sum from bin (m+t); consecutive suffix ends
           differ by exactly the gathered tap value S_ext[m+t].  Tap weights
           are the closed-form single-piece quintics of reference bspline5.
  finale:  global min/max via AllReduce, on-device normalization, clamped
           u8 output.

The d-axis is sharded: core r owns d-planes [16r, 16r+16) == tile row i=r.

Host<->device traffic dominates on the axon-tunneled PJRT path (~50-60
MB/s wire, ~0.1s fixed dispatch cost), so the kernel is shaped around the
wire: x ships once per core as uint16 in natural [16,128,128] layout (the
h-octet blocking for phase 4 is re-staged on device via DRAM-to-DRAM DMA,
the KDE reads the planes directly), the small constants ship packed into
three tensors, and the output is normalized [0,1] emitted as u8.  The JAX
persistent compilation cache removes the per-dispatch NEFF recompile that
run_bass_via_pjrt's fresh-jit-per-call otherwise triggers.
"""

import os
import sys

import numpy as np

sys.path.insert(0, "/opt/trn_rl_repo")

import jax

# smaller NEFF -> faster per-dispatch executable deserialize/load
os.environ.setdefault("CONCOURSE_SCRUB_NEFF_DEBUG_INFO", "1")

for _k, _v in (
    ("jax_compilation_cache_dir", os.path.join("/tmp", "clahe3d_jax_cache")),
    ("jax_persistent_cache_min_entry_size_bytes", -1),
    ("jax_persistent_cache_min_compile_time_secs", 0.0),
):
    try:
        jax.config.update(_k, _v)
    except Exception:
        pass

import concourse.bacc as bacc
import concourse.bass as bass
import concourse.mybir as mybir
import concourse.tile as tile
from concourse.bass_utils import run_bass_kernel_spmd

F32 = mybir.dt.float32
F16 = mybir.dt.float16
U16 = mybir.dt.uint16
U8 = mybir.dt.uint8
AF = mybir.ActivationFunctionType
ALU = mybir.AluOpType
AX = mybir.AxisListType

N_CORES = 8
D = H = W = 128
GD = GH = GW = 8
TD = TH = TW = 16
VPT = TD * TH * TW            # 4096
NB = 64
DS = D // N_CORES             # 16 d-planes per core
NT_OWN = GH * GW              # 64 tiles per core
NPAIR = NT_OWN // 2           # 32 tile pairs in phase 1
BW_KDE = 0.001
EXTW = 74                     # padded S segment width (2+64+2 used, 6 zero)
NSEG = W                      # 128 segments (one per w) per partition
SCAN_N = NSEG * EXTW          # 9472 scanned elements
NBLK = 16                     # h-octet blocks
LIMIT = float(np.floor(4.0 * VPT / NB))   # 256.0


# ----------------------------------------------------------------------------
# host-side constants (float32, mirrors reference.axis_matrix)
# ----------------------------------------------------------------------------
def _bspline5_np(x):
    t = np.abs(np.asarray(x, np.float64))
    w0 = 11.0 / 20.0 - t**2 / 2.0 + t**4 / 4.0 - t**5 / 12.0
    w1 = (17.0 / 40.0 + 5.0 * t / 8.0 - 7.0 * t**2 / 4.0 + 5.0 * t**3 / 4.0
          - 3.0 * t**4 / 8.0 + t**5 / 24.0)
    w2 = (3.0 - t) ** 5 / 120.0
    return np.where(t < 1.0, w0, np.where(t < 2.0, w1, np.where(t < 3.0, w2, 0.0)))


def _axis_matrix_np(size, g):
    c = np.linspace(-0.5 - 0.25 / g, g - 1 + 0.5 + 0.25 / g, size, dtype=np.float32)
    base = np.floor(c).astype(np.int32) - 2
    taps = base[:, None] + np.arange(6)[None, :]
    wgt = _bspline5_np(c[:, None].astype(np.float32)
                       - taps.astype(np.float32)).astype(np.float32)
    i = np.remainder(taps, 2 * g)
    idx = np.where(i < g, i, 2 * g - 1 - i)
    M = np.zeros((size, g), np.float32)
    np.add.at(M, (np.arange(size)[:, None].repeat(6, 1), idx), wgt)
    return M


def _host_constants():
    # D == H == W == 128 and GD == GH == GW == 8, so one axis matrix serves
    # all three axes.
    M = _axis_matrix_np(D, GD)
    mT = np.ascontiguousarray(M.T).astype(np.float32)          # [8, 128]

    consts = {}
    # ohJ[h, j] = [h // 16 == j]: PE partition-reduction of the per-plane
    # KDE partial sums over each tile's 16 h-rows
    ohJ = np.zeros((128, 8), np.float32)
    ohJ[np.arange(128), np.arange(128) // 16] = 1.0
    consts["ohJ"] = ohJ

    # one-hot selectors for the on-device LHS (Kronecker) build:
    #   ohA[k, (i,j)] = [i == k],  ohB[k, (i,j)] = [j == k]
    ii, jj = np.divmod(np.arange(64), 8)
    ohA = np.zeros((8, 64), np.float32)
    ohA[ii, np.arange(64)] = 1.0
    ohB = np.zeros((8, 64), np.float32)
    ohB[jj, np.arange(64)] = 1.0

    # quintic tap-weight coefficients (Horner, highest power first), per tap:
    #   t=0: B5(f+2) = (1-f)^5/120      t=3: B5(1-f)   (w0 piece)
    #   t=1: B5(f+1) (w1 piece)         t=4: B5(2-f)   (w1 piece)
    #   t=2: B5(f)   (w0 piece)         t=5: B5(f-3) = f^5/120
    def poly_from(fn):
        xs = np.linspace(0.0, 1.0, 6)
        V = np.vander(xs, 6, increasing=True)
        c = np.linalg.solve(V, fn(xs))
        return c[::-1]

    polys = [
        poly_from(lambda f: _bspline5_np(f + 2.0)),
        poly_from(lambda f: _bspline5_np(f + 1.0)),
        poly_from(lambda f: _bspline5_np(f)),
        poly_from(lambda f: _bspline5_np(1.0 - f)),
        poly_from(lambda f: _bspline5_np(2.0 - f)),
        poly_from(lambda f: _bspline5_np(f - 3.0)),
    ]
    coef = np.stack(polys, 1).astype(np.float32)          # [6 deg, 6 tap]

    # rows[0, 0:64]  = iota64, rows[0, 64:100] = wb coefs,
    # rows[0, 100:174] = iota74
    rows = np.zeros((1, 174), np.float32)
    rows[0, 0:NB] = np.arange(NB, dtype=np.float32)
    rows[0, NB:NB + 36] = coef.reshape(36)
    rows[0, 100:100 + EXTW] = np.arange(EXTW, dtype=np.float32)
    consts["rows"] = rows

    # m8 packs the [8, .] family: [0:16]=mdT (per core), [16:80]=ohA,
    # [80:144]=ohB, [144:272]=mT
    m8_all = []
    for r in range(N_CORES):
        m8 = np.zeros((8, 272), np.float32)
        m8[:, 0:DS] = mT[:, r * DS:(r + 1) * DS]
        m8[:, 16:80] = ohA
        m8[:, 80:144] = ohB
        m8[:, 144:272] = mT
        m8_all.append(m8)
    return consts, m8_all


# ----------------------------------------------------------------------------
# the Bass program (SPMD; identical on all cores, per-core data via inputs)
# ----------------------------------------------------------------------------
def _build_program():
    nc = bacc.Bacc("TRN2", target_bir_lowering=False, debug=False,
                   num_devices=N_CORES)

    # per-core voxel shard, natural layout
    x_in = nc.dram_tensor("x", [DS, H, W], U16, kind="ExternalInput")
    # output in kernel layout [(d,h8), (blk, w)], u8; host un-permutes
    y_out = nc.dram_tensor("y", [128, NBLK * W], U8, kind="ExternalOutput")

    ohJ_d = nc.dram_tensor("ohJ", [128, 8], F32, kind="ExternalInput")
    m8_d = nc.dram_tensor("m8", [8, 272], F32, kind="ExternalInput")
    rows_d = nc.dram_tensor("rows", [1, 174], F32, kind="ExternalInput")

    s_act = float(np.float32(1.0) / np.float32(BW_KDE))

    with tile.TileContext(nc) as tc:
        with (
            tc.tile_pool(name="const", bufs=1) as cpool,
            tc.tile_pool(name="dram", bufs=1, space="DRAM") as dpool,
            tc.tile_pool(name="p1", bufs=2) as p1,
            tc.tile_pool(name="p1ps", bufs=2, space="PSUM") as p1ps,
            tc.tile_pool(name="small", bufs=2) as sm,
            tc.tile_pool(name="u1ps", bufs=2, space="PSUM") as u1ps,
            tc.tile_pool(name="big", bufs=1) as big,
            tc.tile_pool(name="scan", bufs=1) as scanp,
            tc.tile_pool(name="sx", bufs=1) as sxp,
            tc.tile_pool(name="blk", bufs=2) as blkp,
            tc.tile_pool(name="s2ps", bufs=2, space="PSUM") as s2ps,
        ):
            # ---- collective bounce buffers -------------------------------
            hist_own = dpool.tile([NT_OWN, NB], F32, name="hist_own")
            hist_all = dpool.tile([N_CORES * NT_OWN, NB], F32,
                                  addr_space="Shared", name="hist_all")
            cdf_dram = dpool.tile([512, NB], F32, name="cdf_dram")
            mm_in = dpool.tile([1, 4], F32, name="mm_in")
            mm_min = dpool.tile([1, 4], F32, addr_space="Shared", name="mm_min")
            mm_max = dpool.tile([1, 4], F32, addr_space="Shared", name="mm_max")
            sb_dram = dpool.tile([1, 2], F32, name="sb_dram")

            # ---- constants ----------------------------------------------
            c_ohJ = cpool.tile([128, 8], F32)
            nc.sync.dma_start(c_ohJ[:], ohJ_d[:])
            c_m8 = cpool.tile([8, 272], F32)
            nc.sync.dma_start(c_m8[:], m8_d[:])
            c_mdT = c_m8[:, 0:DS]
            c_ohA = c_m8[:, 16:80]
            c_ohB = c_m8[:, 80:144]
            c_mT = c_m8[:, 144:272]
            c_rows = cpool.tile([128, 174], F32)
            nc.sync.dma_start(c_rows[:], rows_d[:].broadcast_to([128, 174]))
            c_iota64 = c_rows[:, 0:NB]
            c_wbcoef = c_rows[:, NB:NB + 36]
            c_iota74f = c_rows[:, 100:100 + EXTW]

            # ---- on-device LHS build: c_lhs[(ij), (blk,d,h8)] -----------
            #   = Md[16r+d, i] * Mh[blk*8+h8, j]
            c_ma = cpool.tile([8, 128], F32)        # ma[i,(d,h8)] = mdT[i,d]
            nc.scalar.copy(
                c_ma[:].rearrange("p (d h) -> p d h", h=8),
                c_mdT.unsqueeze(2).broadcast_to([8, DS, 8]))
            a_ps = u1ps.tile([64, 128], F32, tag="u1ps", space="PSUM")
            nc.tensor.matmul(a_ps[:], c_ohA, c_ma[:], start=True, stop=True)
            a_sb = cpool.tile([64, 128], F32)       # A[(ij),(d,h8)] = Md[.,i]
            nc.scalar.copy(a_sb[:], a_ps[:])
            b_ps = u1ps.tile([64, 128], F32, tag="u1ps", space="PSUM")
            nc.tensor.matmul(b_ps[:], c_ohB, c_mT, start=True, stop=True)
            b_sb = cpool.tile([64, 128], F32)       # B[(ij),h] = Mh[h,j]
            nc.scalar.copy(b_sb[:], b_ps[:])
            c_lhs = cpool.tile([64, NBLK * 128], F32)
            c_lhs_v = c_lhs[:].rearrange("p (n m) -> p n m", n=NBLK)
            a_v = a_sb[:].rearrange("p (d h) -> p d h", h=8)
            for blk in range(NBLK):
                nc.vector.tensor_tensor(
                    c_lhs_v[:, blk:blk + 1, :].squeeze(1)
                    .rearrange("p (d h) -> p d h", h=8),
                    a_v,
                    b_sb[:, blk * 8:(blk + 1) * 8].unsqueeze(1)
                    .broadcast_to([64, DS, 8]),
                    op=ALU.mult)

            # ---- stage xblocks in DRAM: [(d,h8), (blk,w)] u16 ----------
            xb_dram = dpool.tile([128, NBLK * W], U16, name="xb_st")
            for d in range(DS):
                nc.sync.dma_start(
                    xb_dram[d * 8:(d + 1) * 8, :]
                    .rearrange("h (n w) -> h n w", n=NBLK),
                    x_in[d:d + 1, :, :]
                    .rearrange("o (n h) w -> o n h w", n=NBLK)
                    .squeeze(0).transpose([1, 0, 2]))

            # ---- phase 1: KDE histograms, voxels on h-partitions --------
            # per d-plane: diff[h,(k,b,w)] = x[h,(k,w)] - bin_b  (f32 math,
            # f16 result), then f16 Square/Exp on ACT, w-reduce to f32,
            # accumulate over d; finally PE-reduce h-groups via ohJ.
            c_binsf = cpool.tile([128, NB], F32)
            nc.vector.tensor_scalar(c_binsf[:], c_iota64,
                                    1.0 / float(NB - 1), None, op0=ALU.mult)
            acc1 = p1.tile([128, GW * NB], F32, tag="kacc", bufs=1)
            nc.vector.memset(acc1[:], 0.0)
            KG = 2                                       # k-tiles per group
            for d in range(DS):
                xpu = p1.tile([128, W], U16, tag="xpu")
                nc.sync.dma_start(xpu[:], x_in[d:d + 1, :, :].squeeze(0))
                xf = p1.tile([128, W], F32, tag="xf")
                nc.vector.tensor_scalar(xf[:], xpu[:], 1.0 / 65535.0, None,
                                        op0=ALU.mult)
                for g in range(GW // KG):
                    diff = p1.tile([128, KG * NB * TW], F16, tag="kdiff",
                                   bufs=1)
                    dv = diff[:].rearrange("p (k b w) -> p k b w", k=KG, b=NB)
                    nc.vector.tensor_tensor(
                        dv,
                        xf[:, g * KG * TW:(g + 1) * KG * TW]
                        .rearrange("p (k w) -> p k w", k=KG)
                        .unsqueeze(2).broadcast_to([128, KG, NB, TW]),
                        c_binsf[:].unsqueeze(1).broadcast_to([128, KG, NB])
                        .unsqueeze(3).broadcast_to([128, KG, NB, TW]),
                        op=ALU.subtract)
                    # scale/32 keeps the f16 square finite (<=977);
                    # the 32^2 is folded into the Exp scale
                    nc.scalar.activation(diff[:], diff[:], AF.Square,
                                         bias=0.0, scale=31.25)
                    nc.scalar.activation(diff[:], diff[:], AF.Exp,
                                         bias=0.0, scale=-512.0)
                    red = p1.tile([128, KG * NB], F32, tag="kred", bufs=1)
                    nc.vector.tensor_reduce(
                        red[:].rearrange("p (k b) -> p k b", k=KG),
                        dv, axis=AX.X, op=ALU.add)
                    nc.vector.tensor_tensor(
                        acc1[:, g * KG * NB:(g + 1) * KG * NB],
                        acc1[:, g * KG * NB:(g + 1) * KG * NB],
                        red[:], op=ALU.add)
            hist_ps = p1ps.tile([8, GW * NB], F32, tag="histps", bufs=1,
                                space="PSUM")
            nc.tensor.matmul(hist_ps[:], c_ohJ[:], acc1[:],
                             start=True, stop=True)
            hist_sb2 = sm.tile([8, GW * NB], F32, tag="hist2")
            nc.scalar.copy(hist_sb2[:], hist_ps[:])
            nc.sync.dma_start(
                hist_own[:].rearrange("(j k) b -> j k b", k=GW),
                hist_sb2[:].rearrange("p (k b) -> p k b", k=GW))

            # ---- AllGather ----------------------------------------------
            nc.gpsimd.collective_compute(
                "AllGather", ALU.bypass,
                replica_groups=[list(range(N_CORES))],
                ins=[hist_own[:]], outs=[hist_all[:]])

            # ---- phase 2: clip/redistribute/cdf (all 512 tiles) ---------
            for chunk in range(4):
                hh = sm.tile([128, NB], F32, tag="ph2h")
                nc.sync.dma_start(hh[:],
                                  hist_all[chunk * 128:(chunk + 1) * 128, :])
                ssum = sm.tile([128, 1], F32, tag="ph2s")
                nc.vector.tensor_reduce(ssum[:], hh[:], axis=AX.X, op=ALU.add)
                denom = sm.tile([128, 1], F32, tag="ph2d")
                nc.vector.tensor_scalar(denom[:], ssum[:], 1.0 / VPT, 1e-10,
                                        op0=ALU.mult, op1=ALU.add)
                dinv = sm.tile([128, 1], F32, tag="ph2di")
                nc.vector.reciprocal(dinv[:], denom[:])
                nc.vector.tensor_scalar(hh[:], hh[:], dinv[:], LIMIT,
                                        op0=ALU.mult, op1=ALU.min)
                clip = sm.tile([128, 1], F32, tag="ph2c")
                nc.vector.tensor_reduce(clip[:], hh[:], axis=AX.X, op=ALU.add)
                nc.vector.tensor_scalar(clip[:], clip[:], -1.0, float(VPT),
                                        op0=ALU.mult, op1=ALU.add)
                qq = sm.tile([128, 1], F32, tag="ph2q")
                nc.vector.tensor_scalar(qq[:], clip[:], 1.0 / NB, None,
                                        op0=ALU.mult)
                rq = sm.tile([128, 1], F32, tag="ph2rq")
                nc.vector.tensor_scalar(rq[:], qq[:], 8388608.0, 8388608.0,
                                        op0=ALU.add, op1=ALU.subtract)
                ltq = sm.tile([128, 1], F32, tag="ph2ltq")
                nc.vector.tensor_tensor(ltq[:], qq[:], rq[:], op=ALU.is_lt)
                redist = sm.tile([128, 1], F32, tag="ph2rd")
                nc.vector.tensor_tensor(redist[:], rq[:], ltq[:],
                                        op=ALU.subtract)
                rs64 = sm.tile([128, 1], F32, tag="ph2r64")
                nc.vector.tensor_scalar(rs64[:], redist[:], float(NB), None,
                                        op0=ALU.mult)
                resid = sm.tile([128, 1], F32, tag="ph2r")
                nc.vector.tensor_tensor(resid[:], clip[:], rs64[:],
                                        op=ALU.subtract)
                nc.vector.tensor_scalar(hh[:], hh[:], redist[:], None,
                                        op0=ALU.add)
                lt = sm.tile([128, NB], F32, tag="ph2lt")
                nc.vector.tensor_scalar(lt[:], c_iota64, resid[:], None,
                                        op0=ALU.is_lt)
                nc.vector.tensor_tensor(hh[:], hh[:], lt[:], op=ALU.add)
                zero1 = sm.tile([128, NB], F32, tag="ph2z")
                nc.vector.memset(zero1[:], 0.0)
                cs = sm.tile([128, NB], F32, tag="ph2cs")
                nc.vector.tensor_tensor_scan(cs[:], hh[:], zero1[:], 0.0,
                                             op0=ALU.add, op1=ALU.add)
                nc.vector.tensor_scalar(cs[:], cs[:], float(NB - 1) / VPT,
                                        None, op0=ALU.mult)
                nc.sync.dma_start(cdf_dram[chunk * 128:(chunk + 1) * 128, :],
                                  cs[:])

            # ---- phase 3 stage 1: U1[(ij), (w,b)] -----------------------
            cdf2 = sm.tile([8, 64 * NB], F32, tag="cdf2")
            nc.sync.dma_start(
                cdf2[:].rearrange("p (ij b) -> p ij b", ij=64),
                cdf_dram[:].rearrange("(ij k) b -> k ij b", k=8))
            cdf2v = cdf2[:].rearrange("p (ij b) -> p ij b", ij=64)
            u1 = big.tile([64, W * NB], F32, tag="u1")
            u1v = u1[:].rearrange("p (w b) -> p w b", b=NB)
            for b in range(NB):
                ps = u1ps.tile([64, W], F32, tag="u1ps", space="PSUM")
                nc.tensor.matmul(ps[:], cdf2v[:, :, b:b + 1].squeeze(2),
                                 c_mT, start=True, stop=True)
                nc.scalar.copy(u1v[:, :, b:b + 1], ps[:].unsqueeze(2))

            # ---- phase 3 stage 2 + phase 4, per h-octet block -----------
            omin = sm.tile([128, 1], F32, tag="omin")
            omax = sm.tile([128, 1], F32, tag="omax")
            yall = big.tile([128, NBLK * W], F32, tag="yall")

            for blk in range(NBLK):
                sext = sxp.tile([128, (NSEG + 1) * EXTW], F32, tag="sext")
                sxv = sext[:].rearrange("p (w e) -> p w e", e=EXTW)
                nc.vector.memset(sxv[:, :, 68:EXTW], 0.0)
                nc.vector.memset(sxv[:, NSEG:NSEG + 1, :], 0.0)
                for ch in range(16):
                    ps2 = s2ps.tile([128, 512], F32, tag="s2", space="PSUM")
                    nc.tensor.matmul(ps2[:],
                                     c_lhs_v[:, blk:blk + 1, :].squeeze(1),
                                     u1[:, ch * 512:(ch + 1) * 512],
                                     start=True, stop=True)
                    dst = sxv[:, ch * 8:(ch + 1) * 8, 2:66]
                    nc.scalar.copy(dst,
                                   ps2[:].rearrange("p (w b) -> p w b", b=NB))
                # reflect pad: ext0=S[1],ext1=S[0],ext66=S[63],ext67=S[62]
                nc.scalar.copy(sxv[:, 0:NSEG, 0:1], sxv[:, 0:NSEG, 3:4])
                nc.scalar.copy(sxv[:, 0:NSEG, 1:2], sxv[:, 0:NSEG, 2:3])
                nc.scalar.copy(sxv[:, 0:NSEG, 66:67], sxv[:, 0:NSEG, 65:66])
                nc.scalar.copy(sxv[:, 0:NSEG, 67:68], sxv[:, 0:NSEG, 64:65])

                xb = blkp.tile([128, W], U16, tag="xb", bufs=1)
                nc.sync.dma_start(xb[:], xb_dram[:, blk * W:(blk + 1) * W])
                cb = blkp.tile([128, W], F32, tag="cb", bufs=1)
                nc.vector.tensor_scalar(cb[:], xb[:],
                                        float(NB - 1) / 65535.0, None,
                                        op0=ALU.mult)
                rr = blkp.tile([128, W], F32, tag="rr", bufs=1)
                nc.vector.tensor_scalar(rr[:], cb[:], 8388608.0, 8388608.0,
                                        op0=ALU.add, op1=ALU.subtract)
                ltc = blkp.tile([128, W], F32, tag="ltc", bufs=1)
                nc.vector.tensor_tensor(ltc[:], cb[:], rr[:], op=ALU.is_lt)
                mm = blkp.tile([128, W], F32, tag="mm", bufs=1)
                nc.vector.tensor_tensor(mm[:], rr[:], ltc[:], op=ALU.subtract)
                fr = blkp.tile([128, W], F32, tag="fr", bufs=1)
                nc.vector.tensor_tensor(fr[:], cb[:], mm[:], op=ALU.subtract)
                m6 = blkp.tile([128, W], F32, tag="m6", bufs=1)
                nc.vector.tensor_scalar(m6[:], mm[:], 6.0, None, op0=ALU.add)

                # maskinv[w, q] = (iota_q != m_w + 6), fp16, padded segment
                mask = blkp.tile([128, (NSEG + 1) * EXTW], F16, tag="mask", bufs=1)
                mkv = mask[:].rearrange("p (w e) -> p w e", e=EXTW)
                nc.gpsimd.memset(mkv[:, NSEG:NSEG + 1, :], 1.0)
                nc.vector.tensor_tensor(
                    mkv[:, 0:NSEG, :],
                    c_iota74f.unsqueeze(1).broadcast_to([128, NSEG, EXTW]),
                    m6[:].unsqueeze(2).broadcast_to([128, W, EXTW]),
                    op=ALU.not_equal)

                # 7 masked-reset scans; suffix ends at segment index 73
                tend = blkp.tile([128, 7 * W], F32, tag="tend", bufs=1)
                tview = tend[:].rearrange("p (t w) -> p t w", t=7)
                sbuf = scanp.tile([128, SCAN_N], F32, tag="scanbuf")
                for t in range(7):
                    nc.vector.tensor_tensor_scan(
                        sbuf[:, 0:SCAN_N],
                        mask[:, 6 - t:6 - t + SCAN_N],
                        sext[:, 0:SCAN_N],
                        0.0, op0=ALU.mult, op1=ALU.add)
                    nc.scalar.copy(
                        tview[:, t:t + 1, :],
                        sbuf[:].rearrange("p (w e) -> p w e", e=EXTW)
                        [:, 0:NSEG, 73:74].transpose([0, 2, 1]))

                # taps (6) and quintic weights, batched [128, 6, W]
                taps = blkp.tile([128, 6 * W], F32, tag="taps", bufs=1)
                tp = taps[:].rearrange("p (t w) -> p t w", t=6)
                nc.vector.tensor_tensor(tp, tview[:, 0:6, :],
                                        tview[:, 1:7, :], op=ALU.subtract)
                wbt = blkp.tile([128, 6 * W], F32, tag="wbt", bufs=1)
                wv = wbt[:].rearrange("p (t w) -> p t w", t=6)
                cview = c_wbcoef.rearrange("p (deg t) -> p deg t", deg=6)
                frb = fr[:].unsqueeze(1).broadcast_to([128, 6, W])
                for deg in range(6):
                    coefb = cview[:, deg:deg + 1, :].transpose(
                        [0, 2, 1]).broadcast_to([128, 6, W])
                    if deg == 0:
                        nc.vector.tensor_copy(wv, coefb)
                    else:
                        nc.vector.tensor_tensor(wv, wv, frb, op=ALU.mult)
                        nc.vector.tensor_tensor(wv, wv, coefb, op=ALU.add)
                nc.vector.tensor_tensor(tp, tp, wv, op=ALU.mult)
                # sum 6 taps -> out block (accumulated in SBUF)
                acc = yall[:, blk * W:(blk + 1) * W]
                nc.vector.tensor_tensor(acc,
                                        tp[:, 0:1, :].squeeze(1),
                                        tp[:, 1:2, :].squeeze(1), op=ALU.add)
                for t in range(2, 6):
                    nc.vector.tensor_tensor(acc, acc,
                                            tp[:, t:t + 1, :].squeeze(1),
                                            op=ALU.add)

            # ---- global min / max ---------------------------------------
            nc.vector.tensor_reduce(omin[:], yall[:], axis=AX.X, op=ALU.min)
            nc.vector.tensor_reduce(omax[:], yall[:], axis=AX.X, op=ALU.max)
            gmin = sm.tile([1, 1], F32, tag="gmin")
            gmax = sm.tile([1, 1], F32, tag="gmax")
            negmin = sm.tile([128, 1], F32, tag="negmin")
            nc.vector.tensor_scalar(negmin[:], omin[:], -1.0, None,
                                    op0=ALU.mult)
            nc.gpsimd.tensor_reduce(gmin[:], negmin[:], axis=AX.XYZWC,
                                    op=ALU.max)
            nc.vector.tensor_scalar(gmin[:], gmin[:], -1.0, None,
                                    op0=ALU.mult)
            nc.gpsimd.tensor_reduce(gmax[:], omax[:], axis=AX.XYZWC,
                                    op=ALU.max)
            g4 = sm.tile([1, 4], F32, tag="g4")
            nc.vector.tensor_copy(g4[:], gmin[:].broadcast_to([1, 4]))
            nc.sync.dma_start(mm_in[:], g4[:])
            nc.gpsimd.collective_compute(
                "AllReduce", ALU.min,
                replica_groups=[list(range(N_CORES))],
                ins=[mm_in[:]], outs=[mm_min[:]])
            g4b = sm.tile([1, 4], F32, tag="g4b")
            nc.vector.tensor_copy(g4b[:], gmax[:].broadcast_to([1, 4]))
            nc.sync.dma_start(mm_in[:], g4b[:])
            nc.gpsimd.collective_compute(
                "AllReduce", ALU.max,
                replica_groups=[list(range(N_CORES))],
                ins=[mm_in[:]], outs=[mm_max[:]])

            # normalize: v*inv + (-mn*inv), write f16
            nmn = sm.tile([1, 4], F32, tag="nmn")
            nmx = sm.tile([1, 4], F32, tag="nmx")
            nc.sync.dma_start(nmn[:], mm_min[:])
            nc.sync.dma_start(nmx[:], mm_max[:])
            rng = sm.tile([1, 1], F32, tag="rng")
            nc.vector.tensor_tensor(rng[:], nmx[:, 0:1], nmn[:, 0:1],
                                    op=ALU.subtract)
            nc.vector.tensor_scalar(rng[:], rng[:], 1e-10, None, op0=ALU.add)
            inv = sm.tile([1, 1], F32, tag="inv")
            nc.vector.reciprocal(inv[:], rng[:])
            nbias = sm.tile([1, 1], F32, tag="nbias")
            nc.vector.tensor_tensor(nbias[:], nmn[:, 0:1], inv[:],
                                    op=ALU.mult)
            nc.vector.tensor_scalar(nbias[:], nbias[:], -1.0, None,
                                    op0=ALU.mult)
            sb2 = sm.tile([1, 2], F32, tag="sb2")
            nc.vector.tensor_copy(sb2[:, 0:1], inv[:])
            nc.vector.tensor_copy(sb2[:, 1:2], nbias[:])
            nc.sync.dma_start(sb_dram[:], sb2[:])
            scal_b = sm.tile([128, 2], F32, tag="scalb")
            nc.sync.dma_start(scal_b[:], sb_dram[:].broadcast_to([128, 2]))
            # stage the normalized volume in the (now dead) mask slot,
            # then clamp to [0, 255] and emit u8 — no new SBUF
            yh = blkp.tile([128, (NSEG + 1) * EXTW], F16, tag="mask", bufs=1)
            nc.scalar.activation(yh[:, 0:NBLK * W], yall[:], AF.Identity,
                                 bias=scal_b[:, 1:2], scale=scal_b[:, 0:1])
            nc.vector.tensor_scalar(yh[:, 0:NBLK * W], yh[:, 0:NBLK * W],
                                    255.0, 255.0, op0=ALU.mult, op1=ALU.min)
            yu8 = blkp.tile([128, 512], U8, tag="yu8", bufs=1)
            for qtr in range(4):
                nc.vector.tensor_scalar(yu8[:], yh[:, qtr * 512:(qtr + 1) * 512],
                                        0.0, None, op0=ALU.max)
                nc.sync.dma_start(y_out[:, qtr * 512:(qtr + 1) * 512], yu8[:])

    nc.compile()
    return nc


_PROGRAM_CACHE = {}


def _get_program():
    if "nc" not in _PROGRAM_CACHE:
        _PROGRAM_CACHE["consts"], _PROGRAM_CACHE["m8"] = _host_constants()
        _PROGRAM_CACHE["nc"] = _build_program()
    return (_PROGRAM_CACHE["nc"], _PROGRAM_CACHE["consts"],
            _PROGRAM_CACHE["m8"])


def _prep_in_maps(xv, consts, m8_all):
    xq = (xv * np.float32(65535.0) + np.float32(0.5)).astype(np.uint16)
    in_maps = []
    for r in range(N_CORES):
        m = {"x": xq[r * DS:(r + 1) * DS], "m8": m8_all[r]}
        m.update(consts)
        in_maps.append(m)
    return in_maps


def _unshard_output(res):
    shards = []
    for r in range(N_CORES):
        yr = (res.results[r]["y"].astype(np.float32)
              * np.float32(1.0 / 255.0)).reshape(DS, 8, NBLK, W)
        shards.append(yr.transpose(0, 2, 1, 3).reshape(DS, H, W))
    return np.concatenate(shards, axis=0)


def kernel(**inputs):
    x = np.asarray(inputs["x"], np.float32)
    orig_shape = x.shape
    xv = np.ascontiguousarray(x.reshape(D, H, W))

    nc, consts, m8_all = _get_program()
    in_maps = _prep_in_maps(xv, consts, m8_all)
    # The axon-tunneled device occasionally fails a dispatch transiently
    # (NRT_EXEC_UNIT_UNRECOVERABLE / garbage results) and recovers on the
    # next attempt; retry a few times before giving up.
    last_exc = None
    for attempt in range(4):
        try:
            res = run_bass_kernel_spmd(nc, in_maps,
                                       core_ids=list(range(N_CORES)))
            out = _unshard_output(res)
        except Exception as exc:      # noqa: BLE001 - deliberate retry net
            last_exc = exc
            import time as _time
            _time.sleep(2.0 * (attempt + 1))
            continue
        # output must be a finite, normalized volume
        if np.isfinite(out).all() and -0.01 <= out.min() and out.max() <= 1.01:
            break
    else:
        if last_exc is not None:
            raise last_exc
    return out.reshape(orig_shape).astype(np.float32)


if __name__ == "__main__":
    rng = np.random.default_rng(0)
    x = rng.random((1, 1, D, H, W), dtype=np.float32)
    y = kernel(x=x)
    print("kernel ran; out shape", y.shape, "range", y.min(), y.max())
